# revision 1
# baseline (speedup 1.0000x reference)
"""DiffAttn transformer layer on 8 Trainium2 NeuronCores.

Sharding: token-parallel, no collectives. Core c handles query tokens
[512*(c%4), 512*(c%4+1)) of batch c//4. Each core receives the full 2048
tokens of its batch to (redundantly) compute K/V, plus all weights.

Per-core math (B=2, N=2048, EMB=1024, H=8, HD=64, FF=4096):
  h   = LN(x)              (stats in [t,e], then PE-transpose -> hT [e,t])
  QT  = 0.125*(Wq^T hT_q)  [hd,t] bf16 per head-pair (2 maps of 64 stacked)
  KT  = Wk^T hT            [hd,t] bf16
  V   = hT^T Wv            [t,d]  bf16
  per head: scoresT[k,q] = KT_chunk^T QT (row-packed map pair)
            eT = exp(scoresT)  (no max-sub: logits are O(1) by construction)
            oT[d,q]  += V_chunk^T eT        (psum accumulate over k chunks)
            sums[*,q]+= ones128^T eT        (broadcast row-sums via matmul)
            o = o1/s1 - lam*o2/s2 ; RMS over d via ones-matmul; *subw
  attn = x_q + o_fT^T Wo      (x2, spilled to DRAM)
  FFN (transposed): gT[f,q] = W1^T h2T ; gelu+bias fused on ScalarE;
            out = x2 + gactT^T W2 + b2
LN scale/bias are folded into the weight matrices host-side (exact).
"""

import numpy as np

import concourse.bass as bass
import concourse.bacc as bacc
import concourse.tile as tile
from concourse import mybir
from concourse.alu_op_type import AluOpType
from concourse.masks import make_identity

EMB = 1024
H = 8
HD = 64
FF = 4096
NKV = 2048
NQ = 512
P = 128
ECH = EMB // P      # 8 emb chunks
NTT = NKV // P      # 16 kv token tiles
NTB = 4             # kv token blocks of 512
NQT = NQ // P       # 4 q token tiles
NFT = FF // P       # 32 ff tiles
KC = NKV // P       # 16 k chunks in attention
EPS = 1e-5
DEPTH = 1
LAM_INIT = float(0.8 - 0.6 * np.exp(-0.3 * DEPTH))

F32 = mybir.dt.float32
F32R = mybir.dt.float32r
BF16 = mybir.dt.bfloat16
AF = mybir.ActivationFunctionType
OP = AluOpType


def _r(ap):
    return ap.bitcast(F32R)


def _bcast_ap(dram_t, parts):
    # [n] dram vector -> [parts, n] zero-stride partition broadcast
    ap = dram_t[:]
    return bass.AP(
        tensor=ap.tensor,
        offset=ap.offset,
        ap=[[0, parts], *ap.ap],
    )


def _layernorm_tile(nc, pools, x_ap):
    """LN of one [128, 1024] tile (pure normalize, no scale/bias)."""
    stats_pool, h_pool, eps_t = pools
    stats = stats_pool.tile([P, 2, 6], F32, tag="bnstats", name="stats")
    nc.vector.bn_stats(out=stats[:, 0, :], in_=x_ap[:, 0:512])
    nc.vector.bn_stats(out=stats[:, 1, :], in_=x_ap[:, 512:1024])
    mv = stats_pool.tile([P, 2], F32, tag="bnaggr", name="mv")
    nc.vector.bn_aggr(out=mv, in_=stats)
    # r = rsqrt(var + eps) = exp(-0.5 * ln(var + eps))  (stays in ln/exp set)
    lnv = stats_pool.tile([P, 1], F32, tag="lnv", name="lnv")
    nc.scalar.activation(lnv, mv[:, 1:2], AF.Ln, bias=eps_t, scale=1.0)
    rr = stats_pool.tile([P, 1], F32, tag="rr", name="rr")
    nc.scalar.activation(rr, lnv, AF.Exp, scale=-0.5)
    h_t = h_pool.tile([P, EMB], BF16, tag="h_tile", name="h_t")
    nc.vector.tensor_scalar(
        out=h_t,
        in0=x_ap,
        scalar1=mv[:, 0:1],
        scalar2=rr,
        op0=OP.subtract,
        op1=OP.mult,
    )
    return h_t


def _transpose_into(nc, psp, h_t, dst, tt, ident):
    """PE-transpose h_t [128 t, 1024 e] into dst[:, ec, tt*128:(tt+1)*128]."""
    for g in range(2):  # two groups of 4 emb chunks -> one psum bank each
        pt = psp.tile([P, 4, P], BF16, tag="ps", name="pt")
        for j in range(4):
            ec = g * 4 + j
            nc.tensor.transpose(
                pt[:, j, :],
                h_t[:, ec * P:(ec + 1) * P],
                ident,
            )
        nc.vector.tensor_copy(
            out=dst[:, g * 4:(g + 1) * 4, tt * P:(tt + 1) * P],
            in_=pt,
        )


_ALLOWED_ACT_SETS = {"natural_log_exp_and_others", "gelu_and_others"}
_orig_gat = bacc.get_activation_tables


def _gat_filtered(arch):
    # Hide every other table set so the selector cannot thrash between
    # single-function sets (ln <-> exp alternation costs ~2.7us per switch).
    return {k: (v if k in _ALLOWED_ACT_SETS else set())
            for k, v in _orig_gat(arch).items()}


bacc.get_activation_tables = _gat_filtered


def build_nc():
    nc = bacc.Bacc("TRN2", target_bir_lowering=False)
    x_kv = nc.declare_dram_parameter("x_kv", [NKV, EMB], F32, isOutput=False)
    x_q = nc.declare_dram_parameter("x_q", [NQ, EMB], F32, isOutput=False)
    wq = nc.declare_dram_parameter("wq", [EMB, EMB], BF16, isOutput=False)
    wk = nc.declare_dram_parameter("wk", [EMB, EMB], BF16, isOutput=False)
    wv = nc.declare_dram_parameter("wv", [EMB, EMB], BF16, isOutput=False)
    wo = nc.declare_dram_parameter("wo", [EMB, EMB], BF16, isOutput=False)
    w1 = nc.declare_dram_parameter("w1", [NFT, P, ECH, P], BF16, isOutput=False)
    w2 = nc.declare_dram_parameter("w2", [FF, EMB], BF16, isOutput=False)
    b1p = nc.declare_dram_parameter("b1p", [FF], F32, isOutput=False)
    b2 = nc.declare_dram_parameter("b2", [EMB], F32, isOutput=False)
    rowq = nc.declare_dram_parameter("rowq", [EMB], F32, isOutput=False)
    rowk = nc.declare_dram_parameter("rowk", [EMB], F32, isOutput=False)
    rowv = nc.declare_dram_parameter("rowv", [EMB], F32, isOutput=False)
    subw = nc.declare_dram_parameter("subw", [P], F32, isOutput=False)
    lamn = nc.declare_dram_parameter("lamn", [1], F32, isOutput=False)
    out = nc.declare_dram_parameter("out", [NQ, EMB], F32, isOutput=True)
    x2d = nc.dram_tensor("x2d", [NQ, EMB], F32)

    with tile.TileContext(nc) as tc:
        _build(tc, x_kv, x_q, wq, wk, wv, wo, w1, w2, b1p, b2,
               rowq, rowk, rowv, subw, lamn, out, x2d)
    nc.compile()
    return nc


def _build(tc, x_kv, x_q, wq, wk, wv, wo, w1, w2, b1p, b2,
           rowq, rowk, rowv, subw, lamn, out, x2d):
    nc = tc.nc
    from contextlib import ExitStack
    ctx = ExitStack()
    with ctx:
        # ------------- pools (base of the stack, live whole kernel) -------
        psp = ctx.enter_context(tc.tile_pool(name="psp", bufs=4, space="PSUM"))
        psp2 = ctx.enter_context(tc.tile_pool(name="psp2", bufs=2, space="PSUM"))
        consts = ctx.enter_context(tc.tile_pool(name="consts", bufs=1))
        stats_pool = ctx.enter_context(tc.tile_pool(name="stats", bufs=3))
        x_pool = ctx.enter_context(tc.tile_pool(name="x", bufs=3))
        h_pool = ctx.enter_context(tc.tile_pool(name="h", bufs=2))
        hT_pool = ctx.enter_context(tc.tile_pool(name="hT", bufs=2))
        e_pool = ctx.enter_context(tc.tile_pool(name="eT", bufs=4))
        fin_pool = ctx.enter_context(tc.tile_pool(name="fin", bufs=4))

        # ---------------- constants ----------------
        ones_bf = consts.tile([P, P], BF16)
        nc.gpsimd.memset(ones_bf, 1.0)
        ident = consts.tile([P, P], BF16)
        make_identity(nc, ident)
        rowq_t = consts.tile([P, H], F32)
        nc.gpsimd.dma_start(out=rowq_t, in_=rowq.rearrange("(h p) -> p h", p=P))
        rowk_t = consts.tile([P, H], F32)
        nc.gpsimd.dma_start(out=rowk_t, in_=rowk.rearrange("(h p) -> p h", p=P))
        rowv_bc = consts.tile([P, EMB], F32)
        nc.gpsimd.dma_start(out=rowv_bc, in_=_bcast_ap(rowv, P))
        subw_t = consts.tile([P, 1], F32)
        nc.gpsimd.dma_start(out=subw_t, in_=subw[:, None])
        lamn_t = consts.tile([P, 1], F32)
        nc.gpsimd.dma_start(out=lamn_t, in_=_bcast_ap(lamn, P))
        b1_t = consts.tile([P, NFT], F32)
        nc.gpsimd.dma_start(out=b1_t, in_=b1p.rearrange("(f p) -> p f", p=P))
        b2_bc = consts.tile([P, EMB], F32)
        nc.gpsimd.dma_start(out=b2_bc, in_=_bcast_ap(b2, P))
        eps_t = consts.tile([P, 1], F32)
        nc.gpsimd.memset(eps_t, EPS)
        lnp = (stats_pool, h_pool, eps_t)

        # ---------------- persistent tiles (LIFO stack) ----------------
        h2T, free_h2T = tc.tile([P, ECH, NQ], BF16, name="h2T")
        QT, free_QT = tc.tile([P, H, NQ], BF16, name="QT")     # [hd-pair, h, q]
        KT, free_KT = tc.tile([P, H, NKV], BF16, name="KT")    # [hd-pair, h, t]
        V, free_V = tc.tile([P, NTT, EMB], BF16, name="V")     # [t, tile, d]

        # ============ Phase 1+2: LN(x_q) -> hTq; Q projection ============
        wq_sb, free_wq = tc.tile([P, ECH, EMB], BF16, name="wq_sb")
        nc.sync.dma_start(out=wq_sb, in_=wq.rearrange("(c p) e -> p c e", p=P))
        x_q_r = x_q.rearrange("(tt p) e -> tt p e", p=P)
        hTq = hT_pool.tile([P, ECH, NQ], BF16, tag="hT", name="hTq")
        for tt in range(NQT):
            x_t = x_pool.tile([P, EMB], F32, tag="x_t", name="x_t")
            nc.sync.dma_start(out=x_t, in_=x_q_r[tt])
            h_t = _layernorm_tile(nc, lnp, x_t)
            _transpose_into(nc, psp, h_t, hTq, tt, ident)
        for h in range(H):
            pq = psp.tile([P, NQ], F32, tag="ps", name="pq")
            for ec in range(ECH):
                nc.tensor.matmul(
                    pq,
                    lhsT=wq_sb[:, ec, h * P:(h + 1) * P],
                    rhs=hTq[:, ec, :],
                    start=(ec == 0),
                    stop=(ec == ECH - 1),
                )
            # QT = 0.125*pq + rowq_pre   (rowq pre-scaled by 0.125 on host)
            nc.scalar.activation(QT[:, h, :], pq, AF.Identity,
                                 bias=rowq_t[:, h:h + 1], scale=0.125)
        free_wq()

        # ============ Phase 3/4: LN(x_kv) -> hT; K,V projections ============
        wk_sb, free_wk = tc.tile([P, ECH, EMB], BF16, name="wk_sb")
        nc.sync.dma_start(out=wk_sb, in_=wk.rearrange("(c p) e -> p c e", p=P))
        wv_sb, free_wv = tc.tile([P, ECH, EMB], BF16, name="wv_sb")
        nc.sync.dma_start(out=wv_sb, in_=wv.rearrange("(c p) e -> p c e", p=P))
        x_kv_r = x_kv.rearrange("(tb tt p) e -> tb tt p e", tb=NTB, p=P)
        for tb in range(NTB):
            hT = hT_pool.tile([P, ECH, NQ], BF16, tag="hT", name="hT")
            for tt in range(NTB):
                x_t = x_pool.tile([P, EMB], F32, tag="x_t", name="x_t")
                nc.sync.dma_start(out=x_t, in_=x_kv_r[tb, tt])
                h_t = _layernorm_tile(nc, lnp, x_t)
                _transpose_into(nc, psp, h_t, hT, tt, ident)
            # K-projection for this token block
            for h in range(H):
                pk = psp.tile([P, NQ], F32, tag="ps", name="pk")
                for ec in range(ECH):
                    nc.tensor.matmul(
                        pk,
                        lhsT=wk_sb[:, ec, h * P:(h + 1) * P],
                        rhs=hT[:, ec, :],
                        start=(ec == 0),
                        stop=(ec == ECH - 1),
                    )
                nc.scalar.activation(KT[:, h, tb * NQ:(tb + 1) * NQ], pk,
                                     AF.Identity, bias=rowk_t[:, h:h + 1])
            # V-projection for this token block
            for tt in range(NTB):
                for dc in range(2):
                    pv = psp.tile([P, NQ], F32, tag="ps", name="pv")
                    for ec in range(ECH):
                        nc.tensor.matmul(
                            pv,
                            lhsT=hT[:, ec, tt * P:(tt + 1) * P],
                            rhs=wv_sb[:, ec, dc * NQ:(dc + 1) * NQ],
                            start=(ec == 0),
                            stop=(ec == ECH - 1),
                        )
                    nc.vector.tensor_tensor(
                        out=V[:, tb * NTB + tt, dc * NQ:(dc + 1) * NQ],
                        in0=pv,
                        in1=rowv_bc[:, dc * NQ:(dc + 1) * NQ],
                        op=OP.add,
                    )
        free_wv()
        free_wk()

        # ============ Phase 5: differential attention ============
        # Units = (head, q-half of 256). Both maps of a unit pack into
        # single PSUM banks as [128, 2, 256]; a unit needs 2 accumulator
        # banks, so three units' state fits PSUM and the finalize of unit
        # u is emitted after unit u+1's k-loop (software pipelining) --
        # the PE never waits on the ACT/DVE finalize chain.
        o_fT, free_o_fT = tc.tile([P, H, NQ], BF16, name="o_fT")   # [d, h, q]
        wo_sb, free_wo = tc.tile([P, H, EMB], BF16, name="wo_sb")
        nc.sync.dma_start(out=wo_sb, in_=wo.rearrange("(h p) e -> p h e", p=P))
        NQH = NQ // 2

        def attn_unit(h, qh):
            """k-loop of one (head, q-half) unit; returns psums."""
            qsl = slice(qh * NQH, (qh + 1) * NQH)
            psO = psp.tile([P, 2, NQH], F32, tag="ps", name="psO")
            psS = psp.tile([P, 2, NQH], F32, tag="ps", name="psS")
            for kc in range(KC):
                # [128, 2, 512] = two PSUM banks; map m lands in bank m so the
                # row-packed pair writes different banks (same-bank concurrent
                # PE writes hang TRN2), yet one strided ACT op exps both.
                pS = psp2.tile([P, 2, NQ], F32, tag="ps2", name="pS")
                ksl = slice(kc * P, (kc + 1) * P)
                nc.tensor.matmul(pS[:, 0, 0:NQH], lhsT=KT[0:HD, h, ksl],
                                 rhs=QT[0:HD, h, qsl],
                                 start=True, stop=True, tile_position=(0, 0))
                nc.tensor.matmul(pS[:, 1, 0:NQH], lhsT=KT[HD:P, h, ksl],
                                 rhs=QT[HD:P, h, qsl],
                                 start=True, stop=True, tile_position=(HD, 0))
                e12 = e_pool.tile([P, 2, NQH], BF16, tag="eT", name="e12")
                nc.scalar.activation(e12, pS[:, :, 0:NQH], AF.Exp)
                vsl = V[:, kc, h * P:(h + 1) * P]
                for m in range(2):
                    nc.tensor.matmul(psO[:, m, :], lhsT=vsl, rhs=e12[:, m, :],
                                     start=(kc == 0 and m == 0),
                                     stop=(kc == KC - 1 and m == 1))
                for m in range(2):
                    nc.tensor.matmul(psS[:, m, :], lhsT=ones_bf,
                                     rhs=e12[:, m, :],
                                     start=(kc == 0 and m == 0),
                                     stop=(kc == KC - 1 and m == 1))
            return psO, psS

        def attn_finalize(h, qh, psO, psS):
            qsl = slice(qh * NQH, (qh + 1) * NQH)
            # RMS is scale-invariant (up to eps): normalize
            # o1 - lam*(s1/s2)*o2 instead of o1/s1 - lam*o2/s2.
            la = fin_pool.tile([P, NQH], F32, tag="fin", name="la")
            nc.scalar.activation(la, psS[:, 0, :], AF.Ln)
            lb = fin_pool.tile([P, NQH], F32, tag="fin", name="lb")
            nc.scalar.activation(lb, psS[:, 1, :], AF.Ln)
            ld = fin_pool.tile([P, NQH], F32, tag="fin", name="ld")
            nc.vector.tensor_tensor(out=ld, in0=la, in1=lb, op=OP.subtract)
            gr = fin_pool.tile([P, NQH], F32, tag="fin", name="gr")
            nc.scalar.activation(gr, ld, AF.Exp)
            t2 = fin_pool.tile([P, NQH], F32, tag="fin", name="t2")
            nc.vector.tensor_tensor(out=t2, in0=psO[:, 1, :], in1=gr,
                                    op=OP.mult)
            oc = fin_pool.tile([P, NQH], F32, tag="fin", name="oc")
            nc.vector.scalar_tensor_tensor(
                out=oc, in0=t2, scalar=lamn_t, in1=psO[:, 0, :],
                op0=OP.mult, op1=OP.add,
            )
            osq = fin_pool.tile([P, NQH], BF16, tag="fin", name="osq")
            nc.vector.tensor_tensor(out=osq, in0=oc, in1=oc, op=OP.mult)
            psQt = psp2.tile([P, 2, NQ], F32, tag="ps2", name="psQt")
            psQ = psQt[:, 0, 0:NQH]
            nc.tensor.matmul(psQ, lhsT=ones_bf, rhs=osq, start=True, stop=True)
            tl = fin_pool.tile([P, NQH], F32, tag="fin", name="tl")
            nc.scalar.activation(tl, psQ, AF.Ln, bias=eps_t, scale=1.0 / P)
            rms = fin_pool.tile([P, NQH], F32, tag="fin", name="rms")
            nc.scalar.activation(rms, tl, AF.Exp, scale=-0.5)
            tmp = fin_pool.tile([P, NQH], F32, tag="fin", name="tmp")
            nc.vector.tensor_tensor(out=tmp, in0=oc, in1=rms, op=OP.mult)
            nc.vector.tensor_scalar_mul(o_fT[:, h, qsl], tmp, subw_t)

        prev = None
        for h in range(H):
            for qh in range(2):
                psO, psS = attn_unit(h, qh)
                if prev is not None:
                    attn_finalize(*prev)
                prev = (h, qh, psO, psS)
        attn_finalize(*prev)

        # ====== Phase 6+7: out-projection + residual -> x2d; LN2 -> h2T ======
        # (fused per q-tile so LN2/transposes overlap the next tile's Wo mms)
        x2d_r = x2d.rearrange("(qt p) e -> qt p e", p=P)
        for qt in range(NQT):
            xo = x_pool.tile([P, EMB], F32, tag="x_t", name="xo")
            xr = x_pool.tile([P, EMB], F32, tag="x_t", name="xr")
            nc.sync.dma_start(out=xr, in_=x_q_r[qt])
            for ecc in range(2):
                esl = slice(ecc * NQ, (ecc + 1) * NQ)
                pa = psp.tile([P, NQ], F32, tag="ps", name="pa")
                for h in range(H):
                    nc.tensor.matmul(
                        pa,
                        lhsT=o_fT[:, h, qt * P:(qt + 1) * P],
                        rhs=wo_sb[:, h, esl],
                        start=(h == 0),
                        stop=(h == H - 1),
                    )
                nc.vector.tensor_tensor(out=xo[:, esl], in0=pa,
                                        in1=xr[:, esl], op=OP.add)
            nc.sync.dma_start(out=x2d_r[qt], in_=xo)
            h_t = _layernorm_tile(nc, lnp, xo)
            _transpose_into(nc, psp, h_t, h2T, qt, ident)
        free_wo()
        free_o_fT()
        free_V()
        free_KT()
        free_QT()

        # ============ Phase 8: FFN ============
        gactT, free_gactT = tc.tile([P, NFT, NQ], BF16, name="gactT")
        w2_r = w2.rearrange("(f p) e -> f p e", p=P)
        out_r = out.rearrange("(qt p) e -> qt p e", p=P)
        with tc.tile_pool(name="w1p", bufs=3) as w1_pool, \
             tc.tile_pool(name="w2p", bufs=3) as w2_pool, \
             tc.tile_pool(name="outp", bufs=2) as out_pool:
            for ft in range(NFT):
                w1t = w1_pool.tile([P, ECH, P], BF16, tag="w1t", name="w1t")
                nc.sync.dma_start(out=w1t, in_=w1[ft])
                pg = psp.tile([P, NQ], F32, tag="ps", name="pg")
                for ec in range(ECH):
                    nc.tensor.matmul(
                        pg,
                        lhsT=w1t[:, ec, :],
                        rhs=h2T[:, ec, :],
                        start=(ec == 0),
                        stop=(ec == ECH - 1),
                    )
                nc.scalar.activation(gactT[:, ft, :], pg, AF.Gelu,
                                     bias=b1_t[:, ft:ft + 1])

            # FFN2: two e-half passes; per pass the 4 q-tile accumulators
            # pack into two 2-bank psum tiles. W2 streams by half-columns.
            for ecc in range(2):
                esl = slice(ecc * NQ, (ecc + 1) * NQ)
                pp = [psp2.tile([P, 2, NQ], F32, tag="ps2", name=f"pp{j}")
                      for j in range(2)]
                for ft in range(NFT):
                    w2t = w2_pool.tile([P, NQ], BF16, tag="w2t", name="w2t")
                    nc.sync.dma_start(out=w2t, in_=w2_r[ft][:, esl])
                    for qt in range(NQT):
                        nc.tensor.matmul(
                            pp[qt // 2][:, qt % 2, :],
                            lhsT=gactT[:, ft, qt * P:(qt + 1) * P],
                            rhs=w2t,
                            start=(ft == 0),
                            stop=(ft == NFT - 1),
                        )
                for qt in range(NQT):
                    xr = x_pool.tile([P, EMB], F32, tag="x_t", name="xr2")
                    nc.sync.dma_start(out=xr, in_=x2d_r[qt])
                    o_t = out_pool.tile([P, NQ], F32, tag="o_t", name="o_t")
                    t = out_pool.tile([P, NQ], F32, tag="res_t", name="res_t")
                    nc.vector.tensor_tensor(out=t, in0=pp[qt // 2][:, qt % 2, :],
                                            in1=xr[:, esl], op=OP.add)
                    nc.vector.tensor_tensor(out=o_t, in0=t,
                                            in1=b2_bc[:, esl], op=OP.add)
                    nc.sync.dma_start(out=out_r[qt][:, esl], in_=o_t)
        free_gactT()
        free_h2T()


_NC_CACHE = None


def _get_nc():
    global _NC_CACHE
    if _NC_CACHE is None:
        _NC_CACHE = build_nc()
    return _NC_CACHE


def make_in_maps(x, ln1_w, ln1_b, Wq, Wk, Wv, Wo, lq1, lk1, lq2, lk2,
                 subln_w, ln2_w, ln2_b, W1, b1, W2, b2):
    """Host-side preprocessing + per-core input maps."""
    f32 = np.float32
    x = np.asarray(x, f32)
    d = lambda a: np.asarray(a, np.float64)
    lam = float(np.exp(np.sum(d(lq1) * d(lk1)))
                - np.exp(np.sum(d(lq2) * d(lk2))) + LAM_INIT)
    import ml_dtypes
    bf16 = ml_dtypes.bfloat16
    wq_f = np.ascontiguousarray(d(ln1_w)[:, None] * d(Wq), bf16)
    wk_f = np.ascontiguousarray(d(ln1_w)[:, None] * d(Wk), bf16)
    wv_f = np.ascontiguousarray(d(ln1_w)[:, None] * d(Wv), bf16)
    rowq = np.ascontiguousarray(0.125 * (d(ln1_b) @ d(Wq)), f32)
    rowk = np.ascontiguousarray(d(ln1_b) @ d(Wk), f32)
    rowv = np.ascontiguousarray(d(ln1_b) @ d(Wv), f32)
    w1_f = np.ascontiguousarray(d(ln2_w)[:, None] * d(W1), bf16)
    # pre-tile for contiguous [128, ECH, 128] weight DMAs:
    # w1[(ec p), (ft f)] -> [ft, p, ec, f]
    w1_f = np.ascontiguousarray(
        w1_f.reshape(8, 128, 32, 128).transpose(2, 1, 0, 3))
    b1p = np.ascontiguousarray(d(b1) + d(ln2_b) @ d(W1), f32)
    subw = np.ascontiguousarray(d(subln_w) * (1.0 - LAM_INIT), f32)
    wo_c = np.ascontiguousarray(np.asarray(Wo, np.float64), bf16)
    w2_c = np.ascontiguousarray(np.asarray(W2, np.float64), bf16)
    b2_c = np.ascontiguousarray(np.asarray(b2, f32))
    lamn = np.asarray([-lam], f32)

    shared = dict(wq=wq_f, wk=wk_f, wv=wv_f, wo=wo_c, w1=w1_f, w2=w2_c,
                  b1p=b1p, b2=b2_c, rowq=rowq, rowk=rowk, rowv=rowv,
                  subw=subw, lamn=lamn)
    in_maps = []
    for c in range(8):
        b, qs = divmod(c, 4)
        m = dict(shared)
        m["x_kv"] = np.ascontiguousarray(x[b])
        m["x_q"] = np.ascontiguousarray(x[b, qs * NQ:(qs + 1) * NQ])
        in_maps.append(m)
    return in_maps


def assemble(results):
    outs = [results[c]["out"] for c in range(8)]
    full = np.concatenate(outs, axis=0).reshape(2, NKV, EMB)
    return np.ascontiguousarray(full.astype(np.float32))


def kernel(**inputs):
    from concourse.bass_utils import run_bass_kernel_spmd
    nc = _get_nc()
    in_maps = make_in_maps(**inputs)
    res = run_bass_kernel_spmd(nc, in_maps, core_ids=list(range(8)))
    return assemble(res.results)



# revision 14
# speedup vs baseline: 1.1365x; 1.1365x over previous
"""DiffAttn transformer layer on 8 Trainium2 NeuronCores.

Sharding: token-parallel, no collectives. Core c handles query tokens
[512*(c%4), 512*(c%4+1)) of batch c//4. Each core receives the full 2048
tokens of its batch to (redundantly) compute K/V, plus all weights.

Per-core math (B=2, N=2048, EMB=1024, H=8, HD=64, FF=4096):
  h   = LN(x)              (stats in [t,e], then PE-transpose -> hT [e,t])
  QT  = 0.125*(Wq^T hT_q)  [hd,t] bf16 per head-pair (2 maps of 64 stacked)
  KT  = Wk^T hT            [hd,t] bf16
  V65 = hT^T Wv | 1        [t,kc,h,129] bf16 (ones column rides along)
  per head/q-half unit, kc processed in PAIRS:
    scoresT[k, m, (kc2, q)] -> one exp per pair ([128,2,512] ACT op)
    o[q, m, 129] += e[k,q128]^T [V|1]   (psum accumulate; col 128 = softmax
                                         denominator -- no row-sum matmuls)
    finalize: per-partition scalars only (s1,s2 on col 128):
      oc = o1 - lam*(s1/s2)*o2 ; rms = (mean(oc^2)+eps)^-1/2 (fused DVE
      square+reduce); o_f[q, qt, h, d] = oc*rms  (subw folded into Wo)
  phase 6: PE-transpose o_f -> o_fT [d,h,q]; attn = x_q + o_fT^T Wo; LN2
  FFN: gT[f,q] = W1^T h2T ; gelu+bias on ACT; W2 resident in SBUF
       (prefetched during attention); out = x2 + gactT^T W2 + b2
LN scale/bias, 0.125 q-scale and subln*(1-LAM_INIT) are folded into the
weight matrices host-side (exact: 0.125 is a power of two).
"""

import numpy as np

import concourse.bass as bass
import concourse.bacc as bacc
import concourse.tile as tile
from concourse import mybir
from concourse.alu_op_type import AluOpType
from concourse.masks import make_identity

EMB = 1024
H = 8
HD = 64
FF = 4096
NKV = 2048
NQ = 512
P = 128
ECH = EMB // P      # 8 emb chunks
NTT = NKV // P      # 16 kv token tiles
NTB = 4             # kv token blocks of 512
NQT = NQ // P       # 4 q token tiles
NFT = FF // P       # 32 ff tiles
KC = NKV // P       # 16 k chunks in attention
VW = 130            # per-head V row: 128 dims + ones col at 128 (+pad)
EPS = 1e-5
DEPTH = 1
LAM_INIT = float(0.8 - 0.6 * np.exp(-0.3 * DEPTH))

F32 = mybir.dt.float32
F32R = mybir.dt.float32r
BF16 = mybir.dt.bfloat16
AF = mybir.ActivationFunctionType
OP = AluOpType


def _bcast_ap(dram_t, parts):
    # [n] dram vector -> [parts, n] zero-stride partition broadcast
    ap = dram_t[:]
    return bass.AP(
        tensor=ap.tensor,
        offset=ap.offset,
        ap=[[0, parts], *ap.ap],
    )


def _layernorm_tile(nc, pools, x_ap):
    """LN of one [128, 1024] tile (pure normalize, no scale/bias)."""
    stats_pool, h_pool, eps_t = pools
    stats = stats_pool.tile([P, 2, 6], F32, tag="bnstats", name="stats")
    nc.vector.bn_stats(out=stats[:, 0, :], in_=x_ap[:, 0:512])
    nc.vector.bn_stats(out=stats[:, 1, :], in_=x_ap[:, 512:1024])
    mv = stats_pool.tile([P, 2], F32, tag="bnaggr", name="mv")
    nc.vector.bn_aggr(out=mv, in_=stats)
    # r = rsqrt(var + eps) = exp(-0.5 * ln(var + eps))  (stays in ln/exp set)
    lnv = stats_pool.tile([P, 1], F32, tag="lnv", name="lnv")
    nc.scalar.activation(lnv, mv[:, 1:2], AF.Ln, bias=eps_t, scale=1.0)
    rr = stats_pool.tile([P, 1], F32, tag="rr", name="rr")
    nc.scalar.activation(rr, lnv, AF.Exp, scale=-0.5)
    h_t = h_pool.tile([P, EMB], BF16, tag="h_tile", name="h_t")
    nc.vector.tensor_scalar(
        out=h_t,
        in0=x_ap,
        scalar1=mv[:, 0:1],
        scalar2=rr,
        op0=OP.subtract,
        op1=OP.mult,
    )
    return h_t


def _transpose_into(nc, psp, h_t, dst, tt, ident):
    """PE-transpose h_t [128 t, 1024 e] into dst[:, ec, tt*128:(tt+1)*128]."""
    for g in range(2):  # two groups of 4 emb chunks -> one psum bank each
        pt = psp.tile([P, 4, P], BF16, tag="ps", name="pt")
        for j in range(4):
            ec = g * 4 + j
            nc.tensor.transpose(
                pt[:, j, :],
                h_t[:, ec * P:(ec + 1) * P],
                ident,
            )
        nc.vector.tensor_copy(
            out=dst[:, g * 4:(g + 1) * 4, tt * P:(tt + 1) * P],
            in_=pt,
        )


_ALLOWED_ACT_SETS = {"natural_log_exp_and_others", "gelu_and_others"}
_orig_gat = bacc.get_activation_tables


def _gat_filtered(arch):
    # Hide every other table set so the selector cannot thrash between
    # single-function sets (ln <-> exp alternation costs ~2.7us per switch).
    return {k: (v if k in _ALLOWED_ACT_SETS else set())
            for k, v in _orig_gat(arch).items()}


bacc.get_activation_tables = _gat_filtered


def build_nc():
    nc = bacc.Bacc("TRN2", target_bir_lowering=False)
    x_kv = nc.declare_dram_parameter("x_kv", [NKV, EMB], F32, isOutput=False)
    x_q = nc.declare_dram_parameter("x_q", [NQ, EMB], F32, isOutput=False)
    wq = nc.declare_dram_parameter("wq", [EMB, EMB], BF16, isOutput=False)
    wk = nc.declare_dram_parameter("wk", [EMB, EMB], BF16, isOutput=False)
    wv = nc.declare_dram_parameter("wv", [EMB, EMB], BF16, isOutput=False)
    wo = nc.declare_dram_parameter("wo", [EMB, EMB], BF16, isOutput=False)
    w1 = nc.declare_dram_parameter("w1", [NFT, P, ECH, P], BF16, isOutput=False)
    w2 = nc.declare_dram_parameter("w2", [FF, EMB], BF16, isOutput=False)
    b1p = nc.declare_dram_parameter("b1p", [FF], F32, isOutput=False)
    b2 = nc.declare_dram_parameter("b2", [EMB], F32, isOutput=False)
    rowq = nc.declare_dram_parameter("rowq", [EMB], F32, isOutput=False)
    rowk = nc.declare_dram_parameter("rowk", [EMB], F32, isOutput=False)
    rowv = nc.declare_dram_parameter("rowv", [EMB], F32, isOutput=False)
    lamn = nc.declare_dram_parameter("lamn", [1], F32, isOutput=False)
    out = nc.declare_dram_parameter("out", [NQ, EMB], F32, isOutput=True)
    x2d = nc.dram_tensor("x2d", [NQ, EMB], F32)

    with tile.TileContext(nc) as tc:
        _build(tc, x_kv, x_q, wq, wk, wv, wo, w1, w2, b1p, b2,
               rowq, rowk, rowv, lamn, out, x2d)
    nc.compile()
    return nc


def _build(tc, x_kv, x_q, wq, wk, wv, wo, w1, w2, b1p, b2,
           rowq, rowk, rowv, lamn, out, x2d):
    nc = tc.nc
    from contextlib import ExitStack
    ctx = ExitStack()
    with ctx:
        # ------------- pools (base of the stack, live whole kernel) -------
        psp = ctx.enter_context(tc.tile_pool(name="psp", bufs=4, space="PSUM"))
        psp2 = ctx.enter_context(tc.tile_pool(name="psp2", bufs=2, space="PSUM"))
        consts = ctx.enter_context(tc.tile_pool(name="consts", bufs=1))
        stats_pool = ctx.enter_context(tc.tile_pool(name="stats", bufs=3))
        x_pool = ctx.enter_context(tc.tile_pool(name="x", bufs=3))
        h_pool = ctx.enter_context(tc.tile_pool(name="h", bufs=2))
        hT_pool = ctx.enter_context(tc.tile_pool(name="hT", bufs=2))
        e_pool = ctx.enter_context(tc.tile_pool(name="eT", bufs=4))
        fin_pool = ctx.enter_context(tc.tile_pool(name="fin", bufs=4))

        # ---------------- constants ----------------
        ident = consts.tile([P, P], BF16)
        make_identity(nc, ident)
        rowq_t = consts.tile([P, H], F32)
        nc.gpsimd.dma_start(out=rowq_t, in_=rowq.rearrange("(h p) -> p h", p=P))
        rowk_t = consts.tile([P, H], F32)
        nc.gpsimd.dma_start(out=rowk_t, in_=rowk.rearrange("(h p) -> p h", p=P))
        rowv_bc = consts.tile([P, ECH, P], F32)
        nc.gpsimd.dma_start(out=rowv_bc, in_=_bcast_ap(rowv, P))
        lamn_t = consts.tile([P, 1], F32)
        nc.gpsimd.dma_start(out=lamn_t, in_=_bcast_ap(lamn, P))
        b1_t = consts.tile([P, NFT], F32)
        nc.gpsimd.dma_start(out=b1_t, in_=b1p.rearrange("(f p) -> p f", p=P))
        b2_bc = consts.tile([P, EMB], F32)
        nc.gpsimd.dma_start(out=b2_bc, in_=_bcast_ap(b2, P))
        eps_t = consts.tile([P, 1], F32)
        nc.gpsimd.memset(eps_t, EPS)
        lnp = (stats_pool, h_pool, eps_t)

        # ---------------- persistent tiles (LIFO stack) ----------------
        # o_f/wo_sb/o_fT sit at the bottom so QT/KT/V65 can be freed right
        # after attention, making room for the SBUF-resident W2 in the FFN.
        h2T, free_h2T = tc.tile([P, ECH, NQ], BF16, name="h2T")
        o_f, free_o_f = tc.tile([P, NQT, H, P], BF16, name="o_f")  # [q,qt,h,d]
        wo_sb, free_wo = tc.tile([P, H, EMB], BF16, name="wo_sb")
        o_fT, free_o_fT = tc.tile([P, H, NQ], BF16, name="o_fT")   # [d, h, q]
        QT, free_QT = tc.tile([P, H, NQ], BF16, name="QT")     # [hd-pair, h, q]
        KT, free_KT = tc.tile([P, H, NKV], BF16, name="KT")    # [hd-pair, h, t]
        # V65: [t, kc-tile, h, 130]: cols 0:128 v-dims, cols 128:130 ones
        # (col 128 is the softmax-denominator column; 129 is alignment pad)
        V, free_V = tc.tile([P, NTT, H, VW], BF16, name="V65")
        nc.vector.memset(V[:, :, :, 128:130], 1.0)

        # ============ Phase 1+2: LN(x_q) -> hTq; Q projection ============
        wq_sb, free_wq = tc.tile([P, ECH, EMB], BF16, name="wq_sb")
        nc.sync.dma_start(out=wq_sb, in_=wq.rearrange("(c p) e -> p c e", p=P))
        x_q_r = x_q.rearrange("(tt p) e -> tt p e", p=P)
        hTq = hT_pool.tile([P, ECH, NQ], BF16, tag="hT", name="hTq")
        for tt in range(NQT):
            x_t = x_pool.tile([P, EMB], F32, tag="x_t", name="x_t")
            nc.sync.dma_start(out=x_t, in_=x_q_r[tt])
            h_t = _layernorm_tile(nc, lnp, x_t)
            _transpose_into(nc, psp, h_t, hTq, tt, ident)
        for h in range(H):
            pq = psp.tile([P, NQ], F32, tag="ps", name="pq")
            for ec in range(ECH):
                nc.tensor.matmul(
                    pq,
                    lhsT=wq_sb[:, ec, h * P:(h + 1) * P],
                    rhs=hTq[:, ec, :],
                    start=(ec == 0),
                    stop=(ec == ECH - 1),
                )
            # wq pre-scaled by 0.125 host-side; rowq also pre-scaled
            nc.scalar.activation(QT[:, h, :], pq, AF.Identity,
                                 bias=rowq_t[:, h:h + 1])
        free_wq()

        # ============ Phase 3/4: LN(x_kv) -> hT; K,V projections ============
        wk_sb, free_wk = tc.tile([P, ECH, EMB], BF16, name="wk_sb")
        nc.sync.dma_start(out=wk_sb, in_=wk.rearrange("(c p) e -> p c e", p=P))
        wv_sb, free_wv = tc.tile([P, ECH, EMB], BF16, name="wv_sb")
        nc.sync.dma_start(out=wv_sb, in_=wv.rearrange("(c p) e -> p c e", p=P))
        x_kv_r = x_kv.rearrange("(tb tt p) e -> tb tt p e", tb=NTB, p=P)
        for tb in range(NTB):
            hT = hT_pool.tile([P, ECH, NQ], BF16, tag="hT", name="hT")
            for tt in range(NTB):
                x_t = x_pool.tile([P, EMB], F32, tag="x_t", name="x_t")
                nc.sync.dma_start(out=x_t, in_=x_kv_r[tb, tt])
                h_t = _layernorm_tile(nc, lnp, x_t)
                _transpose_into(nc, psp, h_t, hT, tt, ident)
            # K-projection for this token block
            for h in range(H):
                pk = psp.tile([P, NQ], F32, tag="ps", name="pk")
                for ec in range(ECH):
                    nc.tensor.matmul(
                        pk,
                        lhsT=wk_sb[:, ec, h * P:(h + 1) * P],
                        rhs=hT[:, ec, :],
                        start=(ec == 0),
                        stop=(ec == ECH - 1),
                    )
                nc.scalar.activation(KT[:, h, tb * NQ:(tb + 1) * NQ], pk,
                                     AF.Identity, bias=rowk_t[:, h:h + 1])
            # V-projection for this token block
            for tt in range(NTB):
                for dc in range(2):
                    pv = psp.tile([P, 4, P], F32, tag="ps", name="pv")
                    for ec in range(ECH):
                        nc.tensor.matmul(
                            pv,
                            lhsT=hT[:, ec, tt * P:(tt + 1) * P],
                            rhs=wv_sb[:, ec, dc * NQ:(dc + 1) * NQ],
                            start=(ec == 0),
                            stop=(ec == ECH - 1),
                        )
                    nc.vector.tensor_tensor(
                        out=V[:, tb * NTB + tt, dc * 4:(dc + 1) * 4, 0:P],
                        in0=pv,
                        in1=rowv_bc[:, dc * 4:(dc + 1) * 4, :],
                        op=OP.add,
                    )
        free_wv()
        free_wk()

        # ============ Phase 5: differential attention ============
        # Units = (head, q-half of 256). kc chunks processed in pairs:
        # 4 score matmuls -> one [128,2,512] exp -> 8 oV matmuls of 130 cols
        # each ([V|1] moving operand; col 128 accumulates the softmax
        # denominator, so there are no row-sum matmuls). Output lands
        # q-major: psO_m [q, g, 130]. Unit u's finalize is emitted after
        # unit u+1's k-loop (software pipelining).
        nc.sync.dma_start(out=wo_sb, in_=wo.rearrange("(h p) e -> p h e", p=P))
        NQH = NQ // 2

        def attn_unit(h, qh):
            """k-loop of one (head, q-half) unit; returns psums."""
            qsl = slice(qh * NQH, (qh + 1) * NQH)
            psO = [psp.tile([P, 2, VW], F32, tag="ps", name=f"psO{m}")
                   for m in range(2)]
            for pr in range(KC // 2):
                # scores for a kc PAIR into one [128, 2, 512] tile =
                # two PSUM banks; map m lands in bank m (same-bank
                # concurrent PE writes hang TRN2), kc parity picks the
                # 256-col half. One strided ACT op exps all four.
                pS = psp2.tile([P, 2, NQ], F32, tag="ps2", name="pS")
                for kh in range(2):
                    kc = 2 * pr + kh
                    ksl = slice(kc * P, (kc + 1) * P)
                    csl = slice(kh * NQH, (kh + 1) * NQH)
                    nc.tensor.matmul(pS[:, 0, csl], lhsT=KT[0:HD, h, ksl],
                                     rhs=QT[0:HD, h, qsl],
                                     start=True, stop=True,
                                     tile_position=(0, 0))
                    nc.tensor.matmul(pS[:, 1, csl], lhsT=KT[HD:P, h, ksl],
                                     rhs=QT[HD:P, h, qsl],
                                     start=True, stop=True,
                                     tile_position=(HD, 0))
                e12 = e_pool.tile([P, 2, NQ], BF16, tag="eT", name="e12")
                nc.scalar.activation(e12, pS, AF.Exp)
                for kh in range(2):
                    kc = 2 * pr + kh
                    for m in range(2):
                        for g in range(2):
                            # one accumulation group per psO[m] 2KB region:
                            # start zero-marks the whole region, so the
                            # g=1 slice's first write also lands on zeros
                            nc.tensor.matmul(
                                psO[m][:, g, :],
                                lhsT=e12[:, m, kh * NQH + g * P:
                                         kh * NQH + (g + 1) * P],
                                rhs=V[:, kc, h, :],
                                start=(pr == 0 and kh == 0 and g == 0),
                                stop=(pr == KC // 2 - 1 and kh == 1
                                      and g == 1),
                            )
            return psO

        def attn_finalize(h, qh, psO):
            # o_m = psO[m][:, g, 0:128], s_m = psO[m][:, g, 128] per q-row.
            # RMS is scale-invariant (up to eps): normalize
            # o1 - lam*(s1/s2)*o2 instead of o1/s1 - lam*o2/s2.
            la = fin_pool.tile([P, 2], F32, tag="fs", name="la")
            nc.scalar.activation(la, psO[0][:, :, 128:129], AF.Ln)
            lb = fin_pool.tile([P, 2], F32, tag="fs", name="lb")
            nc.scalar.activation(lb, psO[1][:, :, 128:129], AF.Ln)
            ld = fin_pool.tile([P, 2], F32, tag="fs", name="ld")
            nc.vector.tensor_tensor(out=ld, in0=la, in1=lb, op=OP.subtract)
            gr = fin_pool.tile([P, 2], F32, tag="fs", name="gr")
            nc.scalar.activation(gr, ld, AF.Exp)
            grl = fin_pool.tile([P, 2], F32, tag="fs", name="grl")
            nc.vector.tensor_scalar_mul(grl, gr, lamn_t)
            # DVE can read only one PSUM operand per op: stage o2 to SBUF
            o2c = fin_pool.tile([P, 2, P], F32, tag="fin2", name="o2c")
            nc.vector.tensor_copy(out=o2c, in_=psO[1][:, :, 0:P])
            oc = fin_pool.tile([P, 2, P], F32, tag="fin", name="oc")
            osq = fin_pool.tile([P, 2, P], BF16, tag="fin3", name="osq")
            rsum = fin_pool.tile([P, 2], F32, tag="fs", name="rsum")
            for g in range(2):
                nc.vector.scalar_tensor_tensor(
                    out=oc[:, g, :], in0=o2c[:, g, :],
                    scalar=grl[:, g:g + 1], in1=psO[0][:, g, 0:P],
                    op0=OP.mult, op1=OP.add,
                )
                nc.scalar.activation(osq[:, g, :], oc[:, g, :], AF.Square,
                                     accum_out=rsum[:, g:g + 1])
            tl = fin_pool.tile([P, 2], F32, tag="fs", name="tl")
            nc.scalar.activation(tl, rsum, AF.Ln, bias=eps_t, scale=1.0 / P)
            rms = fin_pool.tile([P, 2], F32, tag="fs", name="rms")
            nc.scalar.activation(rms, tl, AF.Exp, scale=-0.5)
            for g in range(2):
                nc.vector.tensor_scalar_mul(
                    o_f[:, qh * 2 + g, h, :], oc[:, g, :], rms[:, g:g + 1])

        prev = None
        for h in range(H):
            for qh in range(2):
                psO = attn_unit(h, qh)
                if prev is not None:
                    attn_finalize(*prev)
                prev = (h, qh, psO)
        attn_finalize(*prev)

        # QT/KT/V65 are dead once attention is done -- free them now (they
        # sit above o_f/wo_sb/o_fT on the stack) to make room for W2.
        free_V()
        free_KT()
        free_QT()

        # ====== Phase 6+7: transpose o_f -> o_fT; out-projection +
        # residual -> x2d; LN2 -> h2T (fused per q-tile) ======
        x2d_r = x2d.rearrange("(qt p) e -> qt p e", p=P)
        for qt in range(NQT):
            for g4 in range(2):
                pt = psp.tile([P, 4, P], BF16, tag="ps", name="pto")
                for j in range(4):
                    nc.tensor.transpose(
                        pt[:, j, :], o_f[:, qt, g4 * 4 + j, :], ident)
                nc.vector.tensor_copy(
                    out=o_fT[:, g4 * 4:(g4 + 1) * 4, qt * P:(qt + 1) * P],
                    in_=pt,
                )
            xo = x_pool.tile([P, EMB], F32, tag="x_t", name="xo")
            xr = x_pool.tile([P, EMB], F32, tag="x_t", name="xr")
            nc.sync.dma_start(out=xr, in_=x_q_r[qt])
            for ecc in range(2):
                esl = slice(ecc * NQ, (ecc + 1) * NQ)
                pa = psp.tile([P, NQ], F32, tag="ps", name="pa")
                for h in range(H):
                    nc.tensor.matmul(
                        pa,
                        lhsT=o_fT[:, h, qt * P:(qt + 1) * P],
                        rhs=wo_sb[:, h, esl],
                        start=(h == 0),
                        stop=(h == H - 1),
                    )
                nc.vector.tensor_tensor(out=xo[:, esl], in0=pa,
                                        in1=xr[:, esl], op=OP.add)
            nc.sync.dma_start(out=x2d_r[qt], in_=xo)
            h_t = _layernorm_tile(nc, lnp, xo)
            _transpose_into(nc, psp, h_t, h2T, qt, ident)
        free_o_fT()
        free_wo()
        free_o_f()

        # ============ Phase 8: FFN ============
        gactT, free_gactT = tc.tile([P, NFT, NQ], BF16, name="gactT")
        # W2 resident: 8MB in four 2MB DMAs issued here so they hide under
        # FFN1's ~55us of matmuls; FFN2 then runs with zero DMA stalls.
        w2sb, free_w2sb = tc.tile([P, NFT, EMB], BF16, name="w2sb")
        w2_r = w2.rearrange("(f p) e -> f p e", p=P)
        for fq in range(4):
            nc.sync.dma_start(
                out=w2sb[:, fq * 8:(fq + 1) * 8, :],
                in_=w2_r[fq * 8:(fq + 1) * 8].rearrange("f p e -> p f e"),
            )
        out_r = out.rearrange("(qt p) e -> qt p e", p=P)
        with tc.tile_pool(name="w1p", bufs=3) as w1_pool, \
             tc.tile_pool(name="outp", bufs=2) as out_pool:
            for ft in range(NFT):
                w1t = w1_pool.tile([P, ECH, P], BF16, tag="w1t", name="w1t")
                nc.sync.dma_start(out=w1t, in_=w1[ft])
                pg = psp.tile([P, NQ], F32, tag="ps", name="pg")
                for ec in range(ECH):
                    nc.tensor.matmul(
                        pg,
                        lhsT=w1t[:, ec, :],
                        rhs=h2T[:, ec, :],
                        start=(ec == 0),
                        stop=(ec == ECH - 1),
                    )
                nc.scalar.activation(gactT[:, ft, :], pg, AF.Gelu,
                                     bias=b1_t[:, ft:ft + 1])

            # FFN2: two e-half passes; per pass the 4 q-tile accumulators
            # pack into two 2-bank psum tiles. W2 is SBUF-resident.
            for ecc in range(2):
                esl = slice(ecc * NQ, (ecc + 1) * NQ)
                pp = [psp2.tile([P, 2, NQ], F32, tag="ps2", name=f"pp{j}")
                      for j in range(2)]
                for ft in range(NFT):
                    for qt in range(NQT):
                        nc.tensor.matmul(
                            pp[qt // 2][:, qt % 2, :],
                            lhsT=gactT[:, ft, qt * P:(qt + 1) * P],
                            rhs=w2sb[:, ft, esl],
                            start=(ft == 0),
                            stop=(ft == NFT - 1),
                        )
                for qt in range(NQT):
                    xr = x_pool.tile([P, EMB], F32, tag="x_t", name="xr2")
                    nc.sync.dma_start(out=xr, in_=x2d_r[qt])
                    o_t = out_pool.tile([P, NQ], F32, tag="o_t", name="o_t")
                    t = out_pool.tile([P, NQ], F32, tag="res_t", name="res_t")
                    nc.vector.tensor_tensor(out=t, in0=pp[qt // 2][:, qt % 2, :],
                                            in1=xr[:, esl], op=OP.add)
                    nc.vector.tensor_tensor(out=o_t, in0=t,
                                            in1=b2_bc[:, esl], op=OP.add)
                    nc.sync.dma_start(out=out_r[qt][:, esl], in_=o_t)
        free_w2sb()
        free_gactT()
        free_h2T()


_NC_CACHE = None


def _get_nc():
    global _NC_CACHE
    if _NC_CACHE is None:
        _NC_CACHE = build_nc()
    return _NC_CACHE


def make_in_maps(x, ln1_w, ln1_b, Wq, Wk, Wv, Wo, lq1, lk1, lq2, lk2,
                 subln_w, ln2_w, ln2_b, W1, b1, W2, b2):
    """Host-side preprocessing + per-core input maps."""
    f32 = np.float32
    x = np.asarray(x, f32)
    d = lambda a: np.asarray(a, np.float64)
    lam = float(np.exp(np.sum(d(lq1) * d(lk1)))
                - np.exp(np.sum(d(lq2) * d(lk2))) + LAM_INIT)
    import ml_dtypes
    bf16 = ml_dtypes.bfloat16
    wq_f = np.ascontiguousarray(0.125 * d(ln1_w)[:, None] * d(Wq), bf16)
    wk_f = np.ascontiguousarray(d(ln1_w)[:, None] * d(Wk), bf16)
    wv_f = np.ascontiguousarray(d(ln1_w)[:, None] * d(Wv), bf16)
    rowq = np.ascontiguousarray(0.125 * (d(ln1_b) @ d(Wq)), f32)
    rowk = np.ascontiguousarray(d(ln1_b) @ d(Wk), f32)
    rowv = np.ascontiguousarray(d(ln1_b) @ d(Wv), f32)
    w1_f = np.ascontiguousarray(d(ln2_w)[:, None] * d(W1), bf16)
    # pre-tile for contiguous [128, ECH, 128] weight DMAs:
    # w1[(ec p), (ft f)] -> [ft, p, ec, f]
    w1_f = np.ascontiguousarray(
        w1_f.reshape(8, 128, 32, 128).transpose(2, 1, 0, 3))
    b1p = np.ascontiguousarray(d(b1) + d(ln2_b) @ d(W1), f32)
    # subln (and the 1-LAM_INIT factor) folds into Wo's rows
    subw_full = np.tile(d(subln_w) * (1.0 - LAM_INIT), H)
    wo_c = np.ascontiguousarray(subw_full[:, None] * d(Wo), bf16)
    w2_c = np.ascontiguousarray(np.asarray(W2, np.float64), bf16)
    b2_c = np.ascontiguousarray(np.asarray(b2, f32))
    lamn = np.asarray([-lam], f32)

    shared = dict(wq=wq_f, wk=wk_f, wv=wv_f, wo=wo_c, w1=w1_f, w2=w2_c,
                  b1p=b1p, b2=b2_c, rowq=rowq, rowk=rowk, rowv=rowv,
                  lamn=lamn)
    in_maps = []
    for c in range(8):
        b, qs = divmod(c, 4)
        m = dict(shared)
        m["x_kv"] = np.ascontiguousarray(x[b])
        m["x_q"] = np.ascontiguousarray(x[b, qs * NQ:(qs + 1) * NQ])
        in_maps.append(m)
    return in_maps


def assemble(results):
    outs = [results[c]["out"] for c in range(8)]
    full = np.concatenate(outs, axis=0).reshape(2, NKV, EMB)
    return np.ascontiguousarray(full.astype(np.float32))


def kernel(**inputs):
    from concourse.bass_utils import run_bass_kernel_spmd
    nc = _get_nc()
    in_maps = make_in_maps(**inputs)
    res = run_bass_kernel_spmd(nc, in_maps, core_ids=list(range(8)))
    return assemble(res.results)


# revision 18
# speedup vs baseline: 1.1561x; 1.0172x over previous
"""DiffAttn transformer layer on 8 Trainium2 NeuronCores.

Sharding: token-parallel, no collectives. Core c handles query tokens
[512*(c%4), 512*(c%4+1)) of batch c//4. Each core receives the full 2048
tokens of its batch to (redundantly) compute K/V, plus all weights.

Per-core math (B=2, N=2048, EMB=1024, H=8, HD=64, FF=4096):
  h   = LN(x)              (stats in [t,e], then PE-transpose -> hT [e,t])
  QT  = 0.125*(Wq^T hT_q)  [hd,t] bf16 per head-pair (2 maps of 64 stacked)
  KT  = Wk^T hT            [hd,t] bf16
  V65 = hT^T Wv | 1        [t,kc,h,129] bf16 (ones column rides along)
  per head/q-half unit, kc processed in PAIRS:
    scoresT[k, m, (kc2, q)] -> one exp per pair ([128,2,512] ACT op)
    o[q, m, 129] += e[k,q128]^T [V|1]   (psum accumulate; col 128 = softmax
                                         denominator -- no row-sum matmuls)
    finalize: per-partition scalars only (s1,s2 on col 128):
      oc = o1 - lam*(s1/s2)*o2 ; rms = (mean(oc^2)+eps)^-1/2 (fused DVE
      square+reduce); o_f[q, qt, h, d] = oc*rms  (subw folded into Wo)
  phase 6: PE-transpose o_f -> o_fT [d,h,q]; attn = x_q + o_fT^T Wo; LN2
  FFN: gT[f,q] = W1^T h2T ; gelu+bias on ACT; W2 resident in SBUF
       (prefetched during attention); out = x2 + gactT^T W2 + b2
LN scale/bias, 0.125 q-scale and subln*(1-LAM_INIT) are folded into the
weight matrices host-side (exact: 0.125 is a power of two).
"""

import numpy as np

import concourse.bass as bass
import concourse.bacc as bacc
import concourse.tile as tile
from concourse import mybir
from concourse.alu_op_type import AluOpType
from concourse.masks import make_identity

EMB = 1024
H = 8
HD = 64
FF = 4096
NKV = 2048
NQ = 512
P = 128
ECH = EMB // P      # 8 emb chunks
NTT = NKV // P      # 16 kv token tiles
NTB = 4             # kv token blocks of 512
NQT = NQ // P       # 4 q token tiles
NFT = FF // P       # 32 ff tiles
KC = NKV // P       # 16 k chunks in attention
VW = 130            # per-head V row: 128 dims + ones col at 128 (+pad)
EPS = 1e-5
DEPTH = 1
LAM_INIT = float(0.8 - 0.6 * np.exp(-0.3 * DEPTH))

F32 = mybir.dt.float32
F32R = mybir.dt.float32r
BF16 = mybir.dt.bfloat16
AF = mybir.ActivationFunctionType
OP = AluOpType


def _bcast_ap(dram_t, parts):
    # [n] dram vector -> [parts, n] zero-stride partition broadcast
    ap = dram_t[:]
    return bass.AP(
        tensor=ap.tensor,
        offset=ap.offset,
        ap=[[0, parts], *ap.ap],
    )


def _layernorm_tile(nc, pools, x_ap):
    """LN of one [128, 1024] tile (pure normalize, no scale/bias)."""
    stats_pool, h_pool, eps_t = pools
    stats = stats_pool.tile([P, 2, 6], F32, tag="bnstats", name="stats")
    nc.vector.bn_stats(out=stats[:, 0, :], in_=x_ap[:, 0:512])
    nc.vector.bn_stats(out=stats[:, 1, :], in_=x_ap[:, 512:1024])
    mv = stats_pool.tile([P, 2], F32, tag="bnaggr", name="mv")
    nc.vector.bn_aggr(out=mv, in_=stats)
    # r = rsqrt(var + eps) = exp(-0.5 * ln(var + eps))  (stays in ln/exp set)
    lnv = stats_pool.tile([P, 1], F32, tag="lnv", name="lnv")
    nc.scalar.activation(lnv, mv[:, 1:2], AF.Ln, bias=eps_t, scale=1.0)
    rr = stats_pool.tile([P, 1], F32, tag="rr", name="rr")
    nc.scalar.activation(rr, lnv, AF.Exp, scale=-0.5)
    h_t = h_pool.tile([P, EMB], BF16, tag="h_tile", name="h_t")
    nc.vector.tensor_scalar(
        out=h_t,
        in0=x_ap,
        scalar1=mv[:, 0:1],
        scalar2=rr,
        op0=OP.subtract,
        op1=OP.mult,
    )
    return h_t


def _transpose_into(nc, psp, h_t, dst, tt, ident):
    """PE-transpose h_t [128 t, 1024 e] into dst[:, ec, tt*128:(tt+1)*128]."""
    for g in range(2):  # two groups of 4 emb chunks -> one psum bank each
        pt = psp.tile([P, 4, P], BF16, tag="ps", name="pt")
        for j in range(4):
            ec = g * 4 + j
            nc.tensor.transpose(
                pt[:, j, :],
                h_t[:, ec * P:(ec + 1) * P],
                ident,
            )
        nc.vector.tensor_copy(
            out=dst[:, g * 4:(g + 1) * 4, tt * P:(tt + 1) * P],
            in_=pt,
        )


_ALLOWED_ACT_SETS = {"natural_log_exp_and_others", "gelu_and_others"}
_orig_gat = bacc.get_activation_tables


def _gat_filtered(arch):
    # Hide every other table set so the selector cannot thrash between
    # single-function sets (ln <-> exp alternation costs ~2.7us per switch).
    return {k: (v if k in _ALLOWED_ACT_SETS else set())
            for k, v in _orig_gat(arch).items()}


bacc.get_activation_tables = _gat_filtered


def build_nc():
    nc = bacc.Bacc("TRN2", target_bir_lowering=False)
    x_kv = nc.declare_dram_parameter("x_kv", [NKV, EMB], F32, isOutput=False)
    x_q = nc.declare_dram_parameter("x_q", [NQ, EMB], F32, isOutput=False)
    wq = nc.declare_dram_parameter("wq", [EMB, EMB], BF16, isOutput=False)
    wk = nc.declare_dram_parameter("wk", [EMB, EMB], BF16, isOutput=False)
    wv = nc.declare_dram_parameter("wv", [EMB, EMB], BF16, isOutput=False)
    wo = nc.declare_dram_parameter("wo", [EMB, EMB], BF16, isOutput=False)
    w1 = nc.declare_dram_parameter("w1", [NFT, P, ECH, P], BF16, isOutput=False)
    w2 = nc.declare_dram_parameter("w2", [FF, EMB], BF16, isOutput=False)
    b1p = nc.declare_dram_parameter("b1p", [FF], F32, isOutput=False)
    b2 = nc.declare_dram_parameter("b2", [EMB], F32, isOutput=False)
    rowq = nc.declare_dram_parameter("rowq", [EMB], F32, isOutput=False)
    rowk = nc.declare_dram_parameter("rowk", [EMB], F32, isOutput=False)
    rowv = nc.declare_dram_parameter("rowv", [EMB], F32, isOutput=False)
    lamn = nc.declare_dram_parameter("lamn", [1], F32, isOutput=False)
    out = nc.declare_dram_parameter("out", [NQ, EMB], F32, isOutput=True)
    x2d = nc.dram_tensor("x2d", [NQ, EMB], F32)

    with tile.TileContext(nc) as tc:
        _build(tc, x_kv, x_q, wq, wk, wv, wo, w1, w2, b1p, b2,
               rowq, rowk, rowv, lamn, out, x2d)
    nc.compile()
    return nc


def _build(tc, x_kv, x_q, wq, wk, wv, wo, w1, w2, b1p, b2,
           rowq, rowk, rowv, lamn, out, x2d):
    nc = tc.nc
    from contextlib import ExitStack
    ctx = ExitStack()
    with ctx:
        # ------------- pools (base of the stack, live whole kernel) -------
        psp = ctx.enter_context(tc.tile_pool(name="psp", bufs=4, space="PSUM"))
        psp2 = ctx.enter_context(tc.tile_pool(name="psp2", bufs=2, space="PSUM"))
        consts = ctx.enter_context(tc.tile_pool(name="consts", bufs=1))
        stats_pool = ctx.enter_context(tc.tile_pool(name="stats", bufs=3))
        x_pool = ctx.enter_context(tc.tile_pool(name="x", bufs=3))
        h_pool = ctx.enter_context(tc.tile_pool(name="h", bufs=3))
        hT_pool = ctx.enter_context(tc.tile_pool(name="hT", bufs=2))
        e_pool = ctx.enter_context(tc.tile_pool(name="eT", bufs=3))
        fin_pool = ctx.enter_context(tc.tile_pool(name="fin", bufs=4))

        # ---------------- constants ----------------
        ident = consts.tile([P, P], BF16)
        make_identity(nc, ident)
        rowq_t = consts.tile([P, H], F32)
        nc.gpsimd.dma_start(out=rowq_t, in_=rowq.rearrange("(h p) -> p h", p=P))
        rowk_t = consts.tile([P, H], F32)
        nc.gpsimd.dma_start(out=rowk_t, in_=rowk.rearrange("(h p) -> p h", p=P))
        rowv_bc = consts.tile([P, ECH, P], F32)
        nc.gpsimd.dma_start(out=rowv_bc, in_=_bcast_ap(rowv, P))
        lamn_t = consts.tile([P, 1], F32)
        nc.gpsimd.dma_start(out=lamn_t, in_=_bcast_ap(lamn, P))
        b1_t = consts.tile([P, NFT], F32)
        nc.gpsimd.dma_start(out=b1_t, in_=b1p.rearrange("(f p) -> p f", p=P))
        b2_bc = consts.tile([P, EMB], F32)
        nc.gpsimd.dma_start(out=b2_bc, in_=_bcast_ap(b2, P))
        eps_t = consts.tile([P, 1], F32)
        nc.gpsimd.memset(eps_t, EPS)
        lnp = (stats_pool, h_pool, eps_t)

        # ---------------- persistent tiles (LIFO stack) ----------------
        # o_f/wo_sb/o_fT sit at the bottom so QT/KT/V65 can be freed right
        # after attention, making room for the SBUF-resident W2 in the FFN.
        h2T, free_h2T = tc.tile([P, ECH, NQ], BF16, name="h2T")
        o_f, free_o_f = tc.tile([P, NQT, H, P], BF16, name="o_f")  # [q,qt,h,d]
        wo_sb, free_wo = tc.tile([P, H, EMB], BF16, name="wo_sb")
        o_fT, free_o_fT = tc.tile([P, H, NQ], BF16, name="o_fT")   # [d, h, q]
        QT, free_QT = tc.tile([P, H, NQ], BF16, name="QT")     # [hd-pair, h, q]
        KT, free_KT = tc.tile([P, H, NKV], BF16, name="KT")    # [hd-pair, h, t]
        # V65: [t, kc-tile, h, 130]: cols 0:128 v-dims, cols 128:130 ones
        # (col 128 is the softmax-denominator column; 129 is alignment pad)
        V, free_V = tc.tile([P, NTT, H, VW], BF16, name="V65")
        nc.vector.memset(V[:, :, :, 128:130], 1.0)

        # ============ Phase 1+2: LN(x_q) -> hTq; Q projection ============
        wq_sb, free_wq = tc.tile([P, ECH, EMB], BF16, name="wq_sb")
        nc.sync.dma_start(out=wq_sb, in_=wq.rearrange("(c p) e -> p c e", p=P))
        x_q_r = x_q.rearrange("(tt p) e -> tt p e", p=P)
        hTq = hT_pool.tile([P, ECH, NQ], BF16, tag="hT", name="hTq")
        for tt in range(NQT):
            x_t = x_pool.tile([P, EMB], F32, tag="x_t", name="x_t")
            nc.sync.dma_start(out=x_t, in_=x_q_r[tt])
            h_t = _layernorm_tile(nc, lnp, x_t)
            _transpose_into(nc, psp, h_t, hTq, tt, ident)
        for h in range(H):
            pq = psp.tile([P, NQ], F32, tag="ps", name="pq")
            for ec in range(ECH):
                nc.tensor.matmul(
                    pq,
                    lhsT=wq_sb[:, ec, h * P:(h + 1) * P],
                    rhs=hTq[:, ec, :],
                    start=(ec == 0),
                    stop=(ec == ECH - 1),
                )
            # wq pre-scaled by 0.125 host-side; rowq also pre-scaled
            nc.scalar.activation(QT[:, h, :], pq, AF.Identity,
                                 bias=rowq_t[:, h:h + 1])
        free_wq()

        # ============ Phase 3/4: LN(x_kv) -> hT; K,V projections ============
        wk_sb, free_wk = tc.tile([P, ECH, EMB], BF16, name="wk_sb")
        nc.sync.dma_start(out=wk_sb, in_=wk.rearrange("(c p) e -> p c e", p=P))
        wv_sb, free_wv = tc.tile([P, ECH, EMB], BF16, name="wv_sb")
        nc.sync.dma_start(out=wv_sb, in_=wv.rearrange("(c p) e -> p c e", p=P))
        x_kv_r = x_kv.rearrange("(tb tt p) e -> tb tt p e", tb=NTB, p=P)
        for tb in range(NTB):
            hT = hT_pool.tile([P, ECH, NQ], BF16, tag="hT", name="hT")
            for tt in range(NTB):
                x_t = x_pool.tile([P, EMB], F32, tag="x_t", name="x_t")
                nc.sync.dma_start(out=x_t, in_=x_kv_r[tb, tt])
                h_t = _layernorm_tile(nc, lnp, x_t)
                _transpose_into(nc, psp, h_t, hT, tt, ident)
            # K-projection for this token block
            for h in range(H):
                pk = psp.tile([P, NQ], F32, tag="ps", name="pk")
                for ec in range(ECH):
                    nc.tensor.matmul(
                        pk,
                        lhsT=wk_sb[:, ec, h * P:(h + 1) * P],
                        rhs=hT[:, ec, :],
                        start=(ec == 0),
                        stop=(ec == ECH - 1),
                    )
                nc.scalar.activation(KT[:, h, tb * NQ:(tb + 1) * NQ], pk,
                                     AF.Identity, bias=rowk_t[:, h:h + 1])
            # V-projection for this token block
            for tt in range(NTB):
                for dc in range(2):
                    pv = psp.tile([P, 4, P], F32, tag="ps", name="pv")
                    for ec in range(ECH):
                        nc.tensor.matmul(
                            pv,
                            lhsT=hT[:, ec, tt * P:(tt + 1) * P],
                            rhs=wv_sb[:, ec, dc * NQ:(dc + 1) * NQ],
                            start=(ec == 0),
                            stop=(ec == ECH - 1),
                        )
                    nc.vector.tensor_tensor(
                        out=V[:, tb * NTB + tt, dc * 4:(dc + 1) * 4, 0:P],
                        in0=pv,
                        in1=rowv_bc[:, dc * 4:(dc + 1) * 4, :],
                        op=OP.add,
                    )
        free_wv()
        free_wk()

        # ============ Phase 5: differential attention ============
        # Units = (head, q-half of 256). kc chunks processed in pairs:
        # 4 score matmuls -> one [128,2,512] exp -> 8 oV matmuls of 130 cols
        # each ([V|1] moving operand; col 128 accumulates the softmax
        # denominator, so there are no row-sum matmuls). Output lands
        # q-major: psO_m [q, g, 130]. Unit u's finalize is emitted after
        # unit u+1's k-loop (software pipelining).
        nc.sync.dma_start(out=wo_sb, in_=wo.rearrange("(h p) e -> p h e", p=P))
        NQH = NQ // 2

        def attn_unit(h, qh):
            """k-loop of one (head, q-half) unit; returns psums."""
            qsl = slice(qh * NQH, (qh + 1) * NQH)
            psO = [psp.tile([P, 2, VW], F32, tag="ps", name=f"psO{m}")
                   for m in range(2)]
            for pr in range(KC // 2):
                # scores for a kc PAIR into one [128, 2, 512] tile =
                # two PSUM banks; map m lands in bank m (same-bank
                # concurrent PE writes hang TRN2), kc parity picks the
                # 256-col half. One strided ACT op exps all four.
                pS = psp2.tile([P, 2, NQ], F32, tag="ps2", name="pS")
                for kh in range(2):
                    kc = 2 * pr + kh
                    ksl = slice(kc * P, (kc + 1) * P)
                    csl = slice(kh * NQH, (kh + 1) * NQH)
                    nc.tensor.matmul(pS[:, 0, csl], lhsT=KT[0:HD, h, ksl],
                                     rhs=QT[0:HD, h, qsl],
                                     start=True, stop=True,
                                     tile_position=(0, 0))
                    nc.tensor.matmul(pS[:, 1, csl], lhsT=KT[HD:P, h, ksl],
                                     rhs=QT[HD:P, h, qsl],
                                     start=True, stop=True,
                                     tile_position=(HD, 0))
                e12 = e_pool.tile([P, 2, NQ], BF16, tag="eT", name="e12")
                nc.scalar.activation(e12, pS, AF.Exp)
                for kh in range(2):
                    kc = 2 * pr + kh
                    for m in range(2):
                        for g in range(2):
                            # one accumulation group per psO[m] 2KB region:
                            # start zero-marks the whole region, so the
                            # g=1 slice's first write also lands on zeros
                            nc.tensor.matmul(
                                psO[m][:, g, :],
                                lhsT=e12[:, m, kh * NQH + g * P:
                                         kh * NQH + (g + 1) * P],
                                rhs=V[:, kc, h, :],
                                start=(pr == 0 and kh == 0 and g == 0),
                                stop=(pr == KC // 2 - 1 and kh == 1
                                      and g == 1),
                            )
            return psO

        def attn_finalize(h, qh, psO):
            # o_m = psO[m][:, g, 0:128], s_m = psO[m][:, g, 128] per q-row.
            # RMS is scale-invariant (up to eps): normalize
            # o1 - lam*(s1/s2)*o2 instead of o1/s1 - lam*o2/s2.
            la = fin_pool.tile([P, 2], F32, tag="fs", name="la")
            nc.scalar.activation(la, psO[0][:, :, 128:129], AF.Ln)
            lb = fin_pool.tile([P, 2], F32, tag="fs", name="lb")
            nc.scalar.activation(lb, psO[1][:, :, 128:129], AF.Ln)
            ld = fin_pool.tile([P, 2], F32, tag="fs", name="ld")
            nc.vector.tensor_tensor(out=ld, in0=la, in1=lb, op=OP.subtract)
            gr = fin_pool.tile([P, 2], F32, tag="fs", name="gr")
            nc.scalar.activation(gr, ld, AF.Exp)
            grl = fin_pool.tile([P, 2], F32, tag="fs", name="grl")
            nc.vector.tensor_scalar_mul(grl, gr, lamn_t)
            # DVE can read only one PSUM operand per op: stage o2 to SBUF
            o2c = fin_pool.tile([P, 2, P], F32, tag="fin2", name="o2c")
            nc.vector.tensor_copy(out=o2c, in_=psO[1][:, :, 0:P])
            oc = fin_pool.tile([P, 2, P], F32, tag="fin", name="oc")
            osq = fin_pool.tile([P, 2, P], BF16, tag="fin3", name="osq")
            rsum = fin_pool.tile([P, 2], F32, tag="fs", name="rsum")
            for g in range(2):
                nc.vector.scalar_tensor_tensor(
                    out=oc[:, g, :], in0=o2c[:, g, :],
                    scalar=grl[:, g:g + 1], in1=psO[0][:, g, 0:P],
                    op0=OP.mult, op1=OP.add,
                )
                nc.scalar.activation(osq[:, g, :], oc[:, g, :], AF.Square,
                                     accum_out=rsum[:, g:g + 1])
            tl = fin_pool.tile([P, 2], F32, tag="fs", name="tl")
            nc.scalar.activation(tl, rsum, AF.Ln, bias=eps_t, scale=1.0 / P)
            rms = fin_pool.tile([P, 2], F32, tag="fs", name="rms")
            nc.scalar.activation(rms, tl, AF.Exp, scale=-0.5)
            for g in range(2):
                nc.vector.tensor_scalar_mul(
                    o_f[:, qh * 2 + g, h, :], oc[:, g, :], rms[:, g:g + 1])

        prev = None
        for h in range(H):
            for qh in range(2):
                psO = attn_unit(h, qh)
                if prev is not None:
                    attn_finalize(*prev)
                prev = (h, qh, psO)
        attn_finalize(*prev)

        # QT/KT/V65 are dead once attention is done -- free them now (they
        # sit above o_f/wo_sb/o_fT on the stack) to make room for W2.
        free_V()
        free_KT()
        free_QT()

        # ====== Phase 6+7: transpose o_f -> o_fT; out-projection +
        # residual; LN2 -> h2T ======
        # Pass A emits per-qt transposes/outproj/residual/LN-stats; the h2T
        # transposes are deferred to pass B so the PE stream stalls at most
        # once on the LN2 DVE chain (instead of once per q-tile). xb = x +
        # attn + b2 stays SBUF-resident for FFN2's final add (no DRAM
        # round-trip).
        x2d_r = x2d.rearrange("(qt p) e -> qt p e", p=P)
        h_tiles = []
        for qt in range(NQT):
            for g4 in range(2):
                pt = psp.tile([P, 4, P], BF16, tag="ps", name="pto")
                for j in range(4):
                    nc.tensor.transpose(
                        pt[:, j, :], o_f[:, qt, g4 * 4 + j, :], ident)
                nc.vector.tensor_copy(
                    out=o_fT[:, g4 * 4:(g4 + 1) * 4, qt * P:(qt + 1) * P],
                    in_=pt,
                )
            xo = x_pool.tile([P, EMB], F32, tag="x_t", name="xo")
            xr = x_pool.tile([P, EMB], F32, tag="x_t", name="xr")
            nc.sync.dma_start(out=xr, in_=x_q_r[qt])
            for ecc in range(2):
                esl = slice(ecc * NQ, (ecc + 1) * NQ)
                pa = psp.tile([P, NQ], F32, tag="ps", name="pa")
                for h in range(H):
                    nc.tensor.matmul(
                        pa,
                        lhsT=o_fT[:, h, qt * P:(qt + 1) * P],
                        rhs=wo_sb[:, h, esl],
                        start=(h == 0),
                        stop=(h == H - 1),
                    )
                nc.vector.tensor_tensor(out=xo[:, esl], in0=pa,
                                        in1=xr[:, esl], op=OP.add)
            xb = x_pool.tile([P, EMB], F32, tag="x_t", name="xb")
            nc.vector.tensor_tensor(out=xb, in0=xo, in1=b2_bc, op=OP.add)
            nc.sync.dma_start(out=x2d_r[qt], in_=xb)
            h_tiles.append(_layernorm_tile(nc, lnp, xo))
            if qt > 0:
                _transpose_into(nc, psp, h_tiles[qt - 1], h2T, qt - 1, ident)
        _transpose_into(nc, psp, h_tiles[NQT - 1], h2T, NQT - 1, ident)
        free_o_fT()
        free_wo()
        free_o_f()

        # ============ Phase 8: FFN ============
        gactT, free_gactT = tc.tile([P, NFT, NQ], BF16, name="gactT")
        # W2 resident: 8MB in four 2MB DMAs issued here so they hide under
        # FFN1's ~55us of matmuls; FFN2 then runs with zero DMA stalls.
        w2sb, free_w2sb = tc.tile([P, NFT, EMB], BF16, name="w2sb")
        w2_r = w2.rearrange("(f p) e -> f p e", p=P)
        for fq in range(4):
            nc.sync.dma_start(
                out=w2sb[:, fq * 8:(fq + 1) * 8, :],
                in_=w2_r[fq * 8:(fq + 1) * 8].rearrange("f p e -> p f e"),
            )
        out_r = out.rearrange("(qt p) e -> qt p e", p=P)
        with tc.tile_pool(name="w1p", bufs=3) as w1_pool, \
             tc.tile_pool(name="outp", bufs=2) as out_pool:
            for ft in range(NFT):
                w1t = w1_pool.tile([P, ECH, P], BF16, tag="w1t", name="w1t")
                nc.sync.dma_start(out=w1t, in_=w1[ft])
                pg = psp.tile([P, NQ], F32, tag="ps", name="pg")
                for ec in range(ECH):
                    nc.tensor.matmul(
                        pg,
                        lhsT=w1t[:, ec, :],
                        rhs=h2T[:, ec, :],
                        start=(ec == 0),
                        stop=(ec == ECH - 1),
                    )
                nc.scalar.activation(gactT[:, ft, :], pg, AF.Gelu,
                                     bias=b1_t[:, ft:ft + 1])

            # FFN2: two e-half passes; per pass the 4 q-tile accumulators
            # pack into two 2-bank psum tiles. W2 is SBUF-resident.
            for ecc in range(2):
                esl = slice(ecc * NQ, (ecc + 1) * NQ)
                pp = [psp2.tile([P, 2, NQ], F32, tag="ps2", name=f"pp{j}")
                      for j in range(2)]
                for ft in range(NFT):
                    for qt in range(NQT):
                        nc.tensor.matmul(
                            pp[qt // 2][:, qt % 2, :],
                            lhsT=gactT[:, ft, qt * P:(qt + 1) * P],
                            rhs=w2sb[:, ft, esl],
                            start=(ft == 0),
                            stop=(ft == NFT - 1),
                        )
                for qt in range(NQT):
                    xr = x_pool.tile([P, EMB], F32, tag="x_t", name="xr2")
                    nc.sync.dma_start(out=xr, in_=x2d_r[qt])
                    o_t = out_pool.tile([P, NQ], F32, tag="o_t", name="o_t")
                    nc.vector.tensor_tensor(out=o_t,
                                            in0=pp[qt // 2][:, qt % 2, :],
                                            in1=xr[:, esl], op=OP.add)
                    nc.sync.dma_start(out=out_r[qt][:, esl], in_=o_t)
        free_w2sb()
        free_gactT()
        free_h2T()


_NC_CACHE = None


def _get_nc():
    global _NC_CACHE
    if _NC_CACHE is None:
        _NC_CACHE = build_nc()
    return _NC_CACHE


def make_in_maps(x, ln1_w, ln1_b, Wq, Wk, Wv, Wo, lq1, lk1, lq2, lk2,
                 subln_w, ln2_w, ln2_b, W1, b1, W2, b2):
    """Host-side preprocessing + per-core input maps."""
    f32 = np.float32
    x = np.asarray(x, f32)
    d = lambda a: np.asarray(a, np.float64)
    lam = float(np.exp(np.sum(d(lq1) * d(lk1)))
                - np.exp(np.sum(d(lq2) * d(lk2))) + LAM_INIT)
    import ml_dtypes
    bf16 = ml_dtypes.bfloat16
    wq_f = np.ascontiguousarray(0.125 * d(ln1_w)[:, None] * d(Wq), bf16)
    wk_f = np.ascontiguousarray(d(ln1_w)[:, None] * d(Wk), bf16)
    wv_f = np.ascontiguousarray(d(ln1_w)[:, None] * d(Wv), bf16)
    rowq = np.ascontiguousarray(0.125 * (d(ln1_b) @ d(Wq)), f32)
    rowk = np.ascontiguousarray(d(ln1_b) @ d(Wk), f32)
    rowv = np.ascontiguousarray(d(ln1_b) @ d(Wv), f32)
    w1_f = np.ascontiguousarray(d(ln2_w)[:, None] * d(W1), bf16)
    # pre-tile for contiguous [128, ECH, 128] weight DMAs:
    # w1[(ec p), (ft f)] -> [ft, p, ec, f]
    w1_f = np.ascontiguousarray(
        w1_f.reshape(8, 128, 32, 128).transpose(2, 1, 0, 3))
    b1p = np.ascontiguousarray(d(b1) + d(ln2_b) @ d(W1), f32)
    # subln (and the 1-LAM_INIT factor) folds into Wo's rows
    subw_full = np.tile(d(subln_w) * (1.0 - LAM_INIT), H)
    wo_c = np.ascontiguousarray(subw_full[:, None] * d(Wo), bf16)
    w2_c = np.ascontiguousarray(np.asarray(W2, np.float64), bf16)
    b2_c = np.ascontiguousarray(np.asarray(b2, f32))
    lamn = np.asarray([-lam], f32)

    shared = dict(wq=wq_f, wk=wk_f, wv=wv_f, wo=wo_c, w1=w1_f, w2=w2_c,
                  b1p=b1p, b2=b2_c, rowq=rowq, rowk=rowk, rowv=rowv,
                  lamn=lamn)
    in_maps = []
    for c in range(8):
        b, qs = divmod(c, 4)
        m = dict(shared)
        m["x_kv"] = np.ascontiguousarray(x[b])
        m["x_q"] = np.ascontiguousarray(x[b, qs * NQ:(qs + 1) * NQ])
        in_maps.append(m)
    return in_maps


def assemble(results):
    outs = [results[c]["out"] for c in range(8)]
    full = np.concatenate(outs, axis=0).reshape(2, NKV, EMB)
    return np.ascontiguousarray(full.astype(np.float32))


def kernel(**inputs):
    from concourse.bass_utils import run_bass_kernel_spmd
    nc = _get_nc()
    in_maps = make_in_maps(**inputs)
    res = run_bass_kernel_spmd(nc, in_maps, core_ids=list(range(8)))
    return assemble(res.results)
